# revision 55
# baseline (speedup 1.0000x reference)
"""nn_Attention_77876347011151 — Bass/Tile TRN2 kernel, data-parallel over batch.

Full inputs in, full output out. Shapes hardcoded per spec:
x [8,1025,768], alibi [1,12,1025,1025], coords [8,1024,2], mask [8,1025],
gamma/beta [768], W_qkv [768,2304], W_out [768,768].

Per-core (1 batch element each):
  LN (stats fp32) -> zn bf16 -> PE-transpose to znT [768,1025]
  qkvT = W'^T @ znT  (feature-major, q/k deinterleaved + scaled via host W prep)
  v natural [tok,768] via separate matmul (znT as weights); V'=V*mask
  RoPE on qT/kT rows via stacked cos/±sin tables (host-built)
  scores^T[k,q] = kTz^T@qT per head (K zero-padded to 128); queries 0:1024 only
  e^T = exp(scores^T) * expAlibiT (host-precomputed exp(alibi))
  out_augT[128,q] = [V'|m-replicated]^T @ e^T -> rows 0:64 numerator,
  rows 64:128 denominator replicated; normalize; out = outT^T @ W_out.
Query token 1024 is finished on the host from device dumps of q/K/V'
(fp32, exact same layouts), so all device attention tiles are uniform 2x512.
Keys padded to 1152 (zero k-cols, zero expA pad, zero V-aug pad rows).
"""

import numpy as np
import ml_dtypes

BF16 = ml_dtypes.bfloat16
B, N, D = 8, 1025, 768
H, DH, HALF = 12, 64, 32
NP = N - 1          # patch tokens
NQ = 1024           # queries handled on device
NK = 1152           # padded key count (9*128)
KC = NK // 128      # 9 key chunks
TC_SIZES = [128] * 8 + [1]          # token chunks for LN/v (all 1025 tokens)
QC_SIZES = [512, 512]               # query chunks (device)
OC_SIZES = [128] * 8                # out-proj token chunks (tokens 0:1024)
ROPE_BASE = 8192.0
LN_EPS = 1e-5
SCALE = DH ** -0.5

_CACHE = {}
LAST_RESULTS = None


def _chunks(sizes):
    off = 0
    out = []
    for s in sizes:
        out.append((off, s))
        off += s
    return out


TCS = _chunks(TC_SIZES)
QCS = _chunks(QC_SIZES)
OCS = _chunks(OC_SIZES)


def _build_program():
    import concourse.bass as bass
    import concourse.tile as tile
    from concourse import mybir
    from concourse.masks import make_identity
    from concourse.tile import add_dep_helper
    from contextlib import ExitStack

    dt = mybir.dt
    AF = mybir.ActivationFunctionType

    nc = bass.Bass("TRN2", target_bir_lowering=False, debug=False, num_devices=8)

    x_d = nc.dram_tensor("x", [N, D], dt.float32, kind="ExternalInput").ap()
    alibiT_d = nc.dram_tensor("alibiT", [H, NK, NQ], dt.bfloat16, kind="ExternalInput").ap()
    wqkv_d = nc.dram_tensor("wqkv", [D, 3 * D], dt.bfloat16, kind="ExternalInput").ap()
    wout_d = nc.dram_tensor("wout", [D, D], dt.bfloat16, kind="ExternalInput").ap()
    cvec_d = nc.dram_tensor("cvec", [3 * D, 1], dt.float32, kind="ExternalInput").ap()
    maskcol_d = nc.dram_tensor("maskcol", [NK, 1], dt.float32, kind="ExternalInput").ap()
    c4_d = nc.dram_tensor("c4", [128, NP], dt.bfloat16, kind="ExternalInput").ap()
    s4_d = nc.dram_tensor("s4", [128, NP], dt.bfloat16, kind="ExternalInput").ap()
    clsP_d = nc.dram_tensor("clsP", [128, 128], dt.bfloat16, kind="ExternalInput").ap()
    out_d = nc.dram_tensor("out", [NQ, D], dt.float32, kind="ExternalOutput").ap()
    qdump_d = nc.dram_tensor("qdump", [D, 1], dt.bfloat16, kind="ExternalOutput").ap()
    kdump_d = nc.dram_tensor("kdump", [D, N], dt.bfloat16, kind="ExternalOutput").ap()
    vdump_d = nc.dram_tensor("vdump", [NK, D], dt.bfloat16, kind="ExternalOutput").ap()

    with tile.TileContext(nc) as tc:
        with ExitStack() as ctx:
            # ---- persistent pools ----
            singles = ctx.enter_context(tc.tile_pool(name="singles", bufs=1))
            qT_p = ctx.enter_context(tc.tile_pool(name="qT", bufs=6))
            kTz_p = ctx.enter_context(tc.tile_pool(name="kTz", bufs=12))
            vaug_p = ctx.enter_context(tc.tile_pool(name="vaug", bufs=9))
            outT_p = ctx.enter_context(tc.tile_pool(name="outT", bufs=6))
            wout_p = ctx.enter_context(tc.tile_pool(name="wout", bufs=6))

            ident = singles.tile([128, 128], dt.bfloat16)
            make_identity(nc, ident)
            eps_t = singles.tile([128, 1], dt.float32)
            nc.vector.memset(eps_t, LN_EPS)
            c4_t = singles.tile([128, NP], dt.bfloat16)
            nc.sync.dma_start(out=c4_t, in_=c4_d)
            s4_t = singles.tile([128, NP], dt.bfloat16)
            nc.sync.dma_start(out=s4_t, in_=s4_d)
            maskcol_t = singles.tile([128, KC], dt.float32)
            nc.sync.dma_start(
                out=maskcol_t,
                in_=maskcol_d.rearrange("(c p) o -> p (c o)", p=128),
            )
            cvec_t = singles.tile([128, 18], dt.float32)
            nc.sync.dma_start(
                out=cvec_t, in_=cvec_d.rearrange("(m p) o -> p (m o)", p=128)
            )
            clsP_t = singles.tile([128, 128], dt.bfloat16)
            nc.sync.dma_start(out=clsP_t, in_=clsP_d)

            wout_t = []
            for k in range(6):
                w = wout_p.tile([128, D], dt.bfloat16)
                nc.sync.dma_start(out=w, in_=wout_d[k * 128:(k + 1) * 128, :])
                wout_t.append(w)

            qT = []
            for m in range(6):
                t = qT_p.tile([128, N], dt.bfloat16)
                qT.append(t)
            # kTz[h]: K=128 zero-padded per-head keys — even heads hold k in
            # rows 0:64 (zeros below), odd heads in rows 64:128 (zeros above),
            # so scores matmuls run full-array K=128 with the full q tile as
            # rhs (the other head's q rows hit zero weights).
            kTz = []
            for h in range(H):
                t = kTz_p.tile([128, NK], dt.bfloat16)
                if h % 2 == 0:
                    nc.gpsimd.memset(t[64:128, :], 0.0)
                    nc.gpsimd.memset(t[0:64, N:NK], 0.0)
                else:
                    nc.gpsimd.memset(t[0:64, :], 0.0)
                    nc.gpsimd.memset(t[64:128, N:NK], 0.0)
                kTz.append(t)
            # per-head block: [V*m (64 cols) | m replicated (64 cols)] so the
            # AV matmul emits numerator rows 0:64 + denominator rows 64:128
            vaug = []
            for kc in range(KC):
                t = vaug_p.tile([128, H * 128], dt.bfloat16)
                vaug.append(t)
            nc.gpsimd.memset(vaug[KC - 1], 0.0)
            outT = []
            for k in range(6):
                t = outT_p.tile([128, NQ], dt.bfloat16)
                outT.append(t)

            # ---- stage A+B: LN + transpose to znT ----
            znT_cm = tc.tile_pool(name="znT", bufs=6)
            znT_pool = znT_cm.__enter__()
            znT = []
            for k in range(6):
                z = znT_pool.tile([128, N], dt.bfloat16)
                znT.append(z)
            with tc.tile_pool(name="ln", bufs=3) as ln_p, \
                 tc.tile_pool(name="lnst", bufs=6) as lnst_p, \
                 tc.tile_pool(name="tpsum", bufs=3, space="PSUM") as tp_p, \
                 tc.tile_pool(name="xn", bufs=3) as xn_p:
                for ti, (toff, tsz) in enumerate(TCS):
                    xt = ln_p.tile([128, D], dt.float32)
                    if tsz < 128:
                        nc.gpsimd.memset(xt, 0.0)
                    nc.sync.dma_start(out=xt[:tsz, :], in_=x_d[toff:toff + tsz, :])
                    stats = lnst_p.tile([128, 3, 6], dt.float32)
                    xg = xt.rearrange("p (g d) -> p g d", g=3)
                    for g in range(3):
                        nc.vector.bn_stats(out=stats[:, g, :], in_=xg[:, g, :])
                    mv = lnst_p.tile([128, 2], dt.float32)
                    nc.vector.bn_aggr(out=mv, in_=stats)
                    rstd = lnst_p.tile([128, 1], dt.float32)
                    nc.scalar.activation(
                        out=rstd, in_=mv[:, 1:2], func=AF.Sqrt,
                        bias=eps_t, scale=1.0,
                    )
                    nc.vector.reciprocal(out=rstd, in_=rstd)
                    xn = xn_p.tile([128, D], dt.bfloat16)
                    nc.vector.tensor_scalar(
                        out=xn, in0=xt,
                        scalar1=mv[:, 0:1], scalar2=rstd,
                        op0=mybir.AluOpType.subtract, op1=mybir.AluOpType.mult,
                    )
                    for k in range(6):
                        ps = tp_p.tile([128, 128], dt.bfloat16)
                        nc.tensor.transpose(
                            out=ps[:, :tsz], in_=xn[:, k * 128:(k + 1) * 128],
                            identity=ident[:, :tsz],
                        )
                        nc.scalar.copy(
                            out=znT[k][:, toff:toff + tsz], in_=ps[:, :tsz]
                        )

            # ---- stage C: qkvT (q,k regions) + v natural ----
            with tc.tile_pool(name="wqkv", bufs=6) as wq_p, \
                 tc.tile_pool(name="qkpsum", bufs=3, space="PSUM") as qk_ps, \
                 tc.tile_pool(name="vpsum", bufs=3, space="PSUM") as v_ps, \
                 tc.tile_pool(name="rope", bufs=4) as rope_p, \
                 tc.tile_pool(name="kT6", bufs=6) as kT6_p:
                wq_t = []
                for k in range(6):
                    w = wq_p.tile([128, 3 * D], dt.bfloat16)
                    nc.sync.dma_start(out=w, in_=wqkv_d[k * 128:(k + 1) * 128, :])
                    wq_t.append(w)
                kT6 = []
                for _ in range(6):
                    kt6 = kT6_p.tile([128, N], dt.bfloat16)
                    kT6.append(kt6)
                # v natural: [tok, 768] with znT chunks as weights
                for ti, (toff, tsz) in enumerate(TCS):
                    kc = ti  # same 128-chunking as keys (ti<9)
                    for half in range(2):
                        ps = v_ps.tile([128, 384], dt.float32)
                        for k in range(6):
                            nc.tensor.matmul(
                                out=ps[:tsz, :],
                                lhsT=znT[k][:, toff:toff + tsz],
                                rhs=wq_t[k][:, 2 * D + half * 384:2 * D + (half + 1) * 384],
                                start=(k == 0), stop=(k == 5),
                            )
                        for h in range(half * 6, half * 6 + 6):
                            nc.vector.tensor_scalar(
                                out=vaug[kc][:tsz, h * 128:h * 128 + 64],
                                in0=ps[:tsz, h * 64 - half * 384:(h + 1) * 64 - half * 384],
                                scalar1=maskcol_t[:tsz, kc:kc + 1], scalar2=None,
                                op0=mybir.AluOpType.mult,
                            )
                    # mask replicated into cols [h*128+64 : h*128+128] for all heads
                    msl = maskcol_t[:tsz, kc:kc + 1]
                    mask_rep = bass.AP(
                        tensor=msl.tensor, offset=msl.offset,
                        ap=[list(msl.ap[0]), [0, H], [0, 64]],
                    )
                    dst = vaug[kc][:tsz, :].rearrange("p (h c) -> p h c", h=H)
                    nc.vector.tensor_copy(dst[:, :, 64:128], mask_rep)


                # q,k: feature-major qkvT[m*128:(m+1)*128, tokens]. Depth-2
                # software pipeline: clsfix/rope of tile i-2 is emitted during
                # tile i's qkv matmuls, so the PE queue never blocks on the
                # ACT copies of the tile it just produced.
                def emit_qkv(m):
                    t = qT[m] if m < 6 else kT6[m - 6]
                    for (qoff, qsz) in ((0, 512), (512, 512), (1024, 1)):
                        ps = qk_ps.tile([128, 512], dt.float32, name=f"qkps{m}_{qoff}", tag="qkps")
                        for k in range(6):
                            nc.tensor.matmul(
                                out=ps[:, :qsz],
                                lhsT=wq_t[k][:, m * 128:(m + 1) * 128],
                                rhs=znT[k][:, qoff:qoff + qsz],
                                start=(k == 0), stop=(k == 5),
                            )
                        nc.scalar.activation(
                            out=t[:, qoff:qoff + qsz], in_=ps[:, :qsz],
                            func=AF.Identity, bias=cvec_t[:, m:m + 1], scale=1.0,
                        )

                def emit_fixrope(m):
                    t = qT[m] if m < 6 else kT6[m - 6]
                    # CLS column: reference dots the *unpermuted* CLS q/k
                    # against roped (deinterleaved) features; undo the host
                    # deinterleave for col 0 via a PE permutation matmul
                    cps = qk_ps.tile([128, 1], dt.float32, name=f"clsps{m}", tag="clsps", bufs=2)
                    nc.tensor.matmul(out=cps, lhsT=clsP_t, rhs=t[:, 0:1])
                    # RoPE (cols 1:N): new = t*C4 + swap(t)*S4sign
                    sw = rope_p.tile([128, NP], dt.bfloat16, tag="sw")
                    nc.gpsimd.dma_start(out=sw[0:32, :], in_=t[32:64, 1:N])
                    nc.gpsimd.dma_start(out=sw[32:64, :], in_=t[0:32, 1:N])
                    nc.gpsimd.dma_start(out=sw[64:96, :], in_=t[96:128, 1:N])
                    nc.gpsimd.dma_start(out=sw[96:128, :], in_=t[64:96, 1:N])
                    ra = rope_p.tile([128, NP], dt.bfloat16, tag="ra")
                    nc.vector.tensor_mul(ra, t[:, 1:N], c4_t)
                    rb = rope_p.tile([128, NP], dt.bfloat16, tag="rb")
                    nc.vector.tensor_mul(rb, sw, s4_t)
                    if m < 6:
                        nc.vector.tensor_add(t[:, 1:N], ra, rb)
                        nc.scalar.copy(out=t[:, 0:1], in_=cps)
                    else:
                        # write roped k + fixed CLS directly into the
                        # zero-padded kTz tiles (no scatter DMAs)
                        h0 = 2 * (m - 6)
                        nc.vector.tensor_add(
                            kTz[h0][0:64, 1:N], ra[0:64, :], rb[0:64, :]
                        )
                        nc.vector.tensor_add(
                            kTz[h0 + 1][64:128, 1:N], ra[64:128, :], rb[64:128, :]
                        )
                        nc.scalar.copy(out=kTz[h0][0:64, 0:1], in_=cps[0:64, :])
                        nc.scalar.copy(
                            out=kTz[h0 + 1][64:128, 0:1], in_=cps[64:128, :]
                        )

                morder = (6, 0, 7, 1, 8, 2, 9, 3, 10, 4, 11, 5)
                for i, m in enumerate(morder):
                    emit_qkv(m)
                    if i >= 2:
                        emit_fixrope(morder[i - 2])
                emit_fixrope(morder[-2])
                emit_fixrope(morder[-1])
            znT_cm.__exit__(None, None, None)

            # ---- stage E/F/G: per-head attention ----
            # alibiT_d holds host-precomputed exp(alibi)^T, so
            # e = exp(scores) * expA — ACT reads PSUM scores directly and the
            # multiply runs in DVE 2x bf16 / GpSimd.
            with tc.tile_pool(name="alibi", bufs=6) as al_p, \
                 tc.tile_pool(name="et", bufs=30) as et_p, \
                 tc.tile_pool(name="esc", bufs=3) as esc_p, \
                 tc.tile_pool(name="spsum", bufs=3, space="PSUM") as s_ps, \
                 tc.tile_pool(name="avpsum", bufs=2, space="PSUM") as av_ps, \
                 tc.tile_pool(name="nrm", bufs=4) as nrm_p:
                # head-0 alibi prefetched before any compute so DMA streams
                # during LN/qkv
                al_pre = {}
                for kc in range(KC):
                    al = al_p.tile([128, NQ], dt.bfloat16, name=f"al0_{kc}", tag="al")
                    nc.gpsimd.dma_start(
                        out=al, in_=alibiT_d[0, kc * 128:(kc + 1) * 128, :]
                    )
                    al_pre[kc] = al

                def emit_av_mm(h, et_tiles, qi):
                    # AV matmuls (denominator = rows 64:128, replicated)
                    qoff, qsz = QCS[qi]
                    ps = av_ps.tile([128, 512], dt.float32, name=f"avps{h}_{qoff}", tag="avps")
                    for kc in range(KC):
                        nc.tensor.matmul(
                            out=ps,
                            lhsT=vaug[kc][:, h * 128:(h + 1) * 128],
                            rhs=et_tiles[kc][:, qoff:qoff + qsz],
                            start=(kc == 0), stop=(kc == KC - 1),
                        )
                    return ps

                def emit_norm(h, qi, ps, after=None):
                    # deferred so the reciprocals sit *behind* the eT mults in
                    # the DVE queue and never delay the AV-gating tiles
                    ot = outT[h // 2]
                    hh = (h % 2) * 64
                    qoff, qsz = QCS[qi]
                    rc = nrm_p.tile([64, 512], dt.float32, tag="rc")
                    ri = nc.vector.reciprocal(out=rc, in_=ps[64:128, :])
                    if after is not None:
                        add_dep_helper(ri.ins, after.ins, sync=False,
                                       reason="recip behind next head's eT mults")
                    nc.vector.tensor_mul(
                        ot[hh:hh + 64, qoff:qoff + qsz], ps[0:64, :], rc
                    )

                last_mult = [None]

                def emit_scores_range(h, kcs, et_tiles):
                    qt = qT[h // 2]
                    kt = kTz[h]
                    for kc in kcs:
                        if h == 0:
                            al = al_pre[kc]
                        else:
                            al = al_p.tile([128, NQ], dt.bfloat16, name=f"al{h}_{kc}", tag="al")
                            nc.sync.dma_start(
                                out=al, in_=alibiT_d[h, kc * 128:(kc + 1) * 128, :]
                            )
                        et = et_p.tile([128, NQ], dt.bfloat16, name=f"et{h}_{kc}", tag="et")
                        et_tiles.append(et)
                        ps = s_ps.tile([128, NQ], dt.float32, name=f"sps{h}_{kc}", tag="sps")
                        for (qoff, qsz) in QCS:
                            nc.tensor.matmul(
                                out=ps[:, qoff:qoff + qsz],
                                lhsT=kt[:, kc * 128:(kc + 1) * 128],
                                rhs=qt[:, qoff:qoff + qsz],
                            )
                        sc = esc_p.tile([128, NQ], dt.bfloat16, name=f"sc{h}_{kc}", tag="sc")
                        if kc == KC - 1:
                            # pad chunk: only key row 0 is live (vaug pad rows
                            # are zero, so eT rows 1:128 are never read)
                            if h < 3:
                                nc.gpsimd.memset(et, 0.0)
                            nc.scalar.activation(
                                out=sc[0:1, :], in_=ps[0:1, :], func=AF.Exp
                            )
                            nc.vector.tensor_mul(et[0:1, :], sc[0:1, :], al[0:1, :])
                        else:
                            nc.scalar.activation(out=sc, in_=ps, func=AF.Exp)
                            # early kc tiles can take the slow Pool engine; the
                            # last-produced ones gate the AV start, keep on DVE
                            eng = nc.gpsimd if kc in (0, 2, 4) else nc.vector
                            last_mult[0] = eng.tensor_mul(et, sc, al)

                # depth-2 pipeline: AV(h-2) runs against eT tiles whose
                # exps finished a full head ago, so the AV stop never waits
                pipe = []
                pss = {}
                for h in range(H):
                    ets = []
                    old = pipe[0] if len(pipe) == 2 else None
                    emit_scores_range(h, range(0, 3), ets)
                    if old is not None:
                        pss[0] = emit_av_mm(old[0], old[1], 0)
                    emit_scores_range(h, range(3, 6), ets)
                    if old is not None:
                        pss[1] = emit_av_mm(old[0], old[1], 1)
                    emit_scores_range(h, range(6, KC), ets)
                    if old is not None:
                        emit_norm(old[0], 0, pss[0], after=last_mult[0])
                        emit_norm(old[0], 1, pss[1], after=last_mult[0])
                        pipe.pop(0)
                    pipe.append((h, ets))
                for (h, ets) in pipe:
                    emit_norm(h, 0, emit_av_mm(h, ets, 0))
                    emit_norm(h, 1, emit_av_mm(h, ets, 1))

            # ---- host-path dumps (no dependents; gpsimd DMA queue so the
            # alibi stream on the sync queue is never blocked) ----
            for m in range(6):
                nc.gpsimd.dma_start(
                    out=qdump_d[m * 128:(m + 1) * 128, :], in_=qT[m][:, NQ:N]
                )
            for h in range(H):
                hh = (h % 2) * 64
                nc.gpsimd.dma_start(
                    out=kdump_d[h * 64:(h + 1) * 64, :],
                    in_=kTz[h][hh:hh + 64, 0:N],
                )
            for kc in range(KC):
                src = vaug[kc].rearrange("p (h c) -> p h c", h=H)
                nc.gpsimd.dma_start(
                    out=vdump_d[kc * 128:(kc + 1) * 128, :].rearrange(
                        "p (h c) -> p h c", h=H),
                    in_=src[:, :, 0:64],
                )

            # ---- stage H: out projection (tokens 0:1024) ----
            with tc.tile_pool(name="opsum", bufs=4, space="PSUM") as o_ps, \
                 tc.tile_pool(name="osb", bufs=2) as osb_p:
                for (toff, tsz) in OCS:
                    ob = osb_p.tile([128, D], dt.float32)
                    for nn2 in range(2):
                        ps = o_ps.tile([128, 384], dt.float32)
                        for k in range(6):
                            nc.tensor.matmul(
                                out=ps,
                                lhsT=outT[k][:, toff:toff + tsz],
                                rhs=wout_t[k][:, nn2 * 384:(nn2 + 1) * 384],
                                start=(k == 0), stop=(k == 5),
                            )
                        nc.scalar.copy(
                            out=ob[:, nn2 * 384:(nn2 + 1) * 384], in_=ps
                        )
                    nc.sync.dma_start(out=out_d[toff:toff + tsz, :], in_=ob)

    _split_oversized_waits(nc)
    return nc


def _split_oversized_waits(nc):
    """Walrus rejects >1 sync wait per instruction; hoist extras onto NoOps."""
    import bass_rust
    for f in nc.m.functions:
        for bb in f.blocks:
            il = bb.instructions
            i = 0
            while i < len(il):
                inst = il[i]
                si = inst.sync_info
                if si is not None and si.on_wait and len(si.on_wait) > 1:
                    waits = list(si.on_wait)
                    inst.sync_info = bass_rust.SyncInfo(
                        on_wait=[waits[-1]], on_update=list(si.on_update)
                    )
                    pos = i
                    for j, w in enumerate(waits[:-1]):
                        n = bass_rust.InstNoOp(name=f"{inst.name}-wsplit{j}")
                        n.engine = inst.engine
                        n.sync_info = bass_rust.SyncInfo(on_wait=[w], on_update=[])
                        il.insert(pos, n)
                        pos += 1
                        i += 1
                i += 1


def _host_prep(x, alibi_bias, coords, mask, gamma, beta, W_qkv, W_out):
    """Build per-core input maps (host-side weight prep + sharding)."""
    x = np.asarray(x, np.float32)
    alibi = np.asarray(alibi_bias, np.float32)[0]          # [H, N, N]
    coords = np.asarray(coords, np.float32)
    mask = np.asarray(mask).astype(np.float32)             # [B, N]
    gamma = np.asarray(gamma, np.float32)
    beta = np.asarray(beta, np.float32)
    W_qkv = np.asarray(W_qkv, np.float32)
    W_out = np.asarray(W_out, np.float32)

    # deinterleave rope pairs in q,k head blocks; fold scale into q; gamma into W
    perm = np.arange(3 * D)
    de = np.concatenate([np.arange(0, DH, 2), np.arange(1, DH, 2)])
    for h in range(H):
        perm[h * DH:(h + 1) * DH] = h * DH + de
        perm[D + h * DH:D + (h + 1) * DH] = D + h * DH + de
    Wp = W_qkv[:, perm].copy()
    Wp[:, :D] *= SCALE
    cvec = (beta @ Wp).astype(np.float32).reshape(3 * D, 1)
    Wp = (gamma[:, None] * Wp).astype(BF16)
    Wo = W_out.astype(BF16)

    # exp(alibi): transpose to [H, key, query 0:1024], pad keys to NK with 0
    # (e = exp(scores)*expA, so pad keys contribute exactly 0)
    alibiT = np.zeros((H, NK, NQ), dtype=BF16)
    alibiT[:, :N, :] = np.exp(alibi[:, :NQ, :]).transpose(0, 2, 1)

    # CLS un-deinterleave permutation as a PE matmul weight:
    # out[hh+2r] = in[hh+r], out[hh+2r+1] = in[hh+32+r]; lhsT[k,m] = P[m,k]
    clsP = np.zeros((128, 128), dtype=BF16)
    for hh in (0, 64):
        for r in range(32):
            clsP[hh + r, hh + 2 * r] = 1
            clsP[hh + 32 + r, hh + 2 * r + 1] = 1

    # rope tables per batch: stacked [cos;cos;cos;cos], [-sin;sin;-sin;sin]
    inv_freq = 1.0 / (ROPE_BASE ** (np.arange(HALF, dtype=np.float32) / HALF))
    in_maps = []
    for b in range(B):
        xy = coords[b, :, 0] + coords[b, :, 1]             # [NP]
        fr = inv_freq[:, None] * xy[None, :]               # [HALF, NP]
        c, s = np.cos(fr), np.sin(fr)
        c4 = np.tile(c, (4, 1)).astype(BF16)               # [128, NP]
        s4 = np.concatenate([-s, s, -s, s], 0).astype(BF16)
        maskcol = np.zeros((NK, 1), np.float32)
        maskcol[:N, 0] = mask[b]
        in_maps.append({
            "x": x[b],
            "clsP": clsP,
            "alibiT": alibiT,
            "wqkv": Wp,
            "wout": Wo,
            "cvec": cvec,
            "maskcol": maskcol,
            "c4": c4,
            "s4": s4,
        })
    return in_maps


def _host_row1024(res, b, alibi, mask, W_out):
    """Finish query token 1024 on host from device dumps (fp32)."""
    r = res.results[b]
    q = r["qdump"][:, 0].astype(np.float32)                # [768] roped q_1024
    K = r["kdump"].astype(np.float32)                      # [768, 1025]
    Vp = r["vdump"][:N, :].astype(np.float32)              # [1025, 768] masked v
    out = np.empty(D, np.float32)
    acc = np.zeros(D, np.float32)
    for h in range(H):
        qh = q[h * DH:(h + 1) * DH]
        Kh = K[h * DH:(h + 1) * DH, :]                     # [64, 1025]
        s = qh @ Kh + alibi[h, NQ, :]                      # [1025]
        e = np.exp(s) * mask                               # masked exp weights
        den = e.sum()
        num = e @ Vp[:, h * DH:(h + 1) * DH]               # [64]
        acc[h * DH:(h + 1) * DH] = num / den
    return acc @ W_out


def kernel(x, alibi_bias, coords, mask, gamma, beta, W_qkv, W_out):
    global LAST_RESULTS
    from concourse.bass_utils import run_bass_kernel_spmd

    if "nc" not in _CACHE:
        _CACHE["nc"] = _build_program()
    nc = _CACHE["nc"]

    in_maps = _host_prep(x, alibi_bias, coords, mask, gamma, beta, W_qkv, W_out)
    res = run_bass_kernel_spmd(nc, in_maps, list(range(B)))
    LAST_RESULTS = res

    alibi = np.asarray(alibi_bias, np.float32)[0]
    maskf = np.asarray(mask).astype(np.float32)
    Wo = np.asarray(W_out, np.float32)
    out = np.empty((B, N, D), dtype=np.float32)
    for b in range(B):
        out[b, :NQ] = res.results[b]["out"]
        out[b, NQ] = _host_row1024(res, b, alibi, maskf[b], Wo)
    return out


# revision 56
# speedup vs baseline: 20001.8785x; 20001.8785x over previous
"""nn_Attention_77876347011151 — Bass/Tile TRN2 kernel, data-parallel over batch.

Full inputs in, full output out. Shapes hardcoded per spec:
x [8,1025,768], alibi [1,12,1025,1025], coords [8,1024,2], mask [8,1025],
gamma/beta [768], W_qkv [768,2304], W_out [768,768].

Per-core (1 batch element each):
  LN (stats fp32) -> zn bf16 -> PE-transpose to znT [768,1025]
  qkvT = W'^T @ znT  (feature-major, q/k deinterleaved + scaled via host W prep)
  v natural [tok,768] via separate matmul (znT as weights); V'=V*mask
  RoPE on qT/kT rows via stacked cos/±sin tables (host-built)
  scores^T[k,q] = kTz^T@qT per head (K zero-padded to 128); queries 0:1024 only
  e^T = exp(scores^T) * expAlibiT (host-precomputed exp(alibi))
  out_augT[128,q] = [V'|m-replicated]^T @ e^T -> rows 0:64 numerator,
  rows 64:128 denominator replicated; normalize; out = outT^T @ W_out.
Query token 1024 is finished on the host from device dumps of q/K/V'
(fp32, exact same layouts), so all device attention tiles are uniform 2x512.
Keys padded to 1152 (zero k-cols, zero expA pad, zero V-aug pad rows).
"""

import numpy as np
import ml_dtypes

BF16 = ml_dtypes.bfloat16
B, N, D = 8, 1025, 768
H, DH, HALF = 12, 64, 32
NP = N - 1          # patch tokens
NQ = 1024           # queries handled on device
NK = 1152           # padded key count (9*128)
KC = NK // 128      # 9 key chunks
TC_SIZES = [128] * 8 + [1]          # token chunks for LN/v (all 1025 tokens)
QC_SIZES = [512, 512]               # query chunks (device)
OC_SIZES = [128] * 8                # out-proj token chunks (tokens 0:1024)
ROPE_BASE = 8192.0
LN_EPS = 1e-5
SCALE = DH ** -0.5

_CACHE = {}
LAST_RESULTS = None


def _chunks(sizes):
    off = 0
    out = []
    for s in sizes:
        out.append((off, s))
        off += s
    return out


TCS = _chunks(TC_SIZES)
QCS = _chunks(QC_SIZES)
OCS = _chunks(OC_SIZES)


def _build_program():
    import concourse.bass as bass
    import concourse.tile as tile
    from concourse import mybir
    from concourse.masks import make_identity
    from concourse.tile import add_dep_helper
    from contextlib import ExitStack

    dt = mybir.dt
    AF = mybir.ActivationFunctionType

    nc = bass.Bass("TRN2", target_bir_lowering=False, debug=False, num_devices=8)

    x_d = nc.dram_tensor("x", [N, D], dt.float32, kind="ExternalInput").ap()
    alibiT_d = nc.dram_tensor("alibiT", [H, NK, NQ], dt.bfloat16, kind="ExternalInput").ap()
    wqkv_d = nc.dram_tensor("wqkv", [D, 3 * D], dt.bfloat16, kind="ExternalInput").ap()
    wout_d = nc.dram_tensor("wout", [D, D], dt.bfloat16, kind="ExternalInput").ap()
    cvec_d = nc.dram_tensor("cvec", [3 * D, 1], dt.float32, kind="ExternalInput").ap()
    maskcol_d = nc.dram_tensor("maskcol", [NK, 1], dt.float32, kind="ExternalInput").ap()
    c4_d = nc.dram_tensor("c4", [128, NP], dt.bfloat16, kind="ExternalInput").ap()
    s4_d = nc.dram_tensor("s4", [128, NP], dt.bfloat16, kind="ExternalInput").ap()
    clsP_d = nc.dram_tensor("clsP", [128, 128], dt.bfloat16, kind="ExternalInput").ap()
    out_d = nc.dram_tensor("out", [NQ, D], dt.float32, kind="ExternalOutput").ap()
    qdump_d = nc.dram_tensor("qdump", [D, 1], dt.bfloat16, kind="ExternalOutput").ap()
    kdump_d = nc.dram_tensor("kdump", [D, N], dt.bfloat16, kind="ExternalOutput").ap()
    vdump_d = nc.dram_tensor("vdump", [NK, D], dt.bfloat16, kind="ExternalOutput").ap()

    with tile.TileContext(nc) as tc:
        with ExitStack() as ctx:
            # ---- persistent pools ----
            singles = ctx.enter_context(tc.tile_pool(name="singles", bufs=1))
            qT_p = ctx.enter_context(tc.tile_pool(name="qT", bufs=6))
            kTz_p = ctx.enter_context(tc.tile_pool(name="kTz", bufs=12))
            vaug_p = ctx.enter_context(tc.tile_pool(name="vaug", bufs=9))
            outT_p = ctx.enter_context(tc.tile_pool(name="outT", bufs=6))
            wout_p = ctx.enter_context(tc.tile_pool(name="wout", bufs=6))

            ident = singles.tile([128, 128], dt.bfloat16)
            make_identity(nc, ident)
            eps_t = singles.tile([128, 1], dt.float32)
            nc.vector.memset(eps_t, LN_EPS)
            c4_t = singles.tile([128, NP], dt.bfloat16)
            nc.sync.dma_start(out=c4_t, in_=c4_d)
            s4_t = singles.tile([128, NP], dt.bfloat16)
            nc.sync.dma_start(out=s4_t, in_=s4_d)
            maskcol_t = singles.tile([128, KC], dt.float32)
            nc.sync.dma_start(
                out=maskcol_t,
                in_=maskcol_d.rearrange("(c p) o -> p (c o)", p=128),
            )
            cvec_t = singles.tile([128, 18], dt.float32)
            nc.sync.dma_start(
                out=cvec_t, in_=cvec_d.rearrange("(m p) o -> p (m o)", p=128)
            )
            clsP_t = singles.tile([128, 128], dt.bfloat16)
            nc.sync.dma_start(out=clsP_t, in_=clsP_d)

            wout_t = []
            for k in range(6):
                w = wout_p.tile([128, D], dt.bfloat16)
                nc.sync.dma_start(out=w, in_=wout_d[k * 128:(k + 1) * 128, :])
                wout_t.append(w)

            qT = []
            for m in range(6):
                t = qT_p.tile([128, N], dt.bfloat16)
                qT.append(t)
            # kTz[h]: K=128 zero-padded per-head keys — even heads hold k in
            # rows 0:64 (zeros below), odd heads in rows 64:128 (zeros above),
            # so scores matmuls run full-array K=128 with the full q tile as
            # rhs (the other head's q rows hit zero weights).
            kTz = []
            for h in range(H):
                t = kTz_p.tile([128, NK], dt.bfloat16)
                if h % 2 == 0:
                    nc.gpsimd.memset(t[64:128, :], 0.0)
                    nc.gpsimd.memset(t[0:64, N:NK], 0.0)
                else:
                    nc.gpsimd.memset(t[0:64, :], 0.0)
                    nc.gpsimd.memset(t[64:128, N:NK], 0.0)
                kTz.append(t)
            # per-head block: [V*m (64 cols) | m replicated (64 cols)] so the
            # AV matmul emits numerator rows 0:64 + denominator rows 64:128
            vaug = []
            for kc in range(KC):
                t = vaug_p.tile([128, H * 128], dt.bfloat16)
                vaug.append(t)
            nc.gpsimd.memset(vaug[KC - 1], 0.0)
            outT = []
            for k in range(6):
                t = outT_p.tile([128, NQ], dt.bfloat16)
                outT.append(t)

            # ---- stage A+B: LN + transpose to znT ----
            znT_cm = tc.tile_pool(name="znT", bufs=6)
            znT_pool = znT_cm.__enter__()
            znT = []
            for k in range(6):
                z = znT_pool.tile([128, N], dt.bfloat16)
                znT.append(z)
            with tc.tile_pool(name="ln", bufs=3) as ln_p, \
                 tc.tile_pool(name="lnst", bufs=6) as lnst_p, \
                 tc.tile_pool(name="tpsum", bufs=3, space="PSUM") as tp_p, \
                 tc.tile_pool(name="xn", bufs=3) as xn_p:
                for ti, (toff, tsz) in enumerate(TCS):
                    xt = ln_p.tile([128, D], dt.float32)
                    if tsz < 128:
                        nc.gpsimd.memset(xt, 0.0)
                    nc.sync.dma_start(out=xt[:tsz, :], in_=x_d[toff:toff + tsz, :])
                    stats = lnst_p.tile([128, 3, 6], dt.float32)
                    xg = xt.rearrange("p (g d) -> p g d", g=3)
                    for g in range(3):
                        nc.vector.bn_stats(out=stats[:, g, :], in_=xg[:, g, :])
                    mv = lnst_p.tile([128, 2], dt.float32)
                    nc.vector.bn_aggr(out=mv, in_=stats)
                    rstd = lnst_p.tile([128, 1], dt.float32)
                    nc.scalar.activation(
                        out=rstd, in_=mv[:, 1:2], func=AF.Sqrt,
                        bias=eps_t, scale=1.0,
                    )
                    nc.vector.reciprocal(out=rstd, in_=rstd)
                    xn = xn_p.tile([128, D], dt.bfloat16)
                    nc.vector.tensor_scalar(
                        out=xn, in0=xt,
                        scalar1=mv[:, 0:1], scalar2=rstd,
                        op0=mybir.AluOpType.subtract, op1=mybir.AluOpType.mult,
                    )
                    for k in range(6):
                        ps = tp_p.tile([128, 128], dt.bfloat16)
                        nc.tensor.transpose(
                            out=ps[:, :tsz], in_=xn[:, k * 128:(k + 1) * 128],
                            identity=ident[:, :tsz],
                        )
                        nc.scalar.copy(
                            out=znT[k][:, toff:toff + tsz], in_=ps[:, :tsz]
                        )

            # ---- stage C: qkvT (q,k regions) + v natural ----
            with tc.tile_pool(name="wqkv", bufs=6) as wq_p, \
                 tc.tile_pool(name="qkpsum", bufs=2, space="PSUM") as qk_ps, \
                 tc.tile_pool(name="vpsum", bufs=3, space="PSUM") as v_ps, \
                 tc.tile_pool(name="rope", bufs=4) as rope_p, \
                 tc.tile_pool(name="kT6", bufs=6) as kT6_p:
                wq_t = []
                for k in range(6):
                    w = wq_p.tile([128, 3 * D], dt.bfloat16)
                    nc.sync.dma_start(out=w, in_=wqkv_d[k * 128:(k + 1) * 128, :])
                    wq_t.append(w)
                kT6 = []
                for _ in range(6):
                    kt6 = kT6_p.tile([128, N], dt.bfloat16)
                    kT6.append(kt6)
                # v natural: [tok, 768] with znT chunks as weights
                for ti, (toff, tsz) in enumerate(TCS):
                    kc = ti  # same 128-chunking as keys (ti<9)
                    for half in range(2):
                        ps = v_ps.tile([128, 384], dt.float32)
                        for k in range(6):
                            nc.tensor.matmul(
                                out=ps[:tsz, :],
                                lhsT=znT[k][:, toff:toff + tsz],
                                rhs=wq_t[k][:, 2 * D + half * 384:2 * D + (half + 1) * 384],
                                start=(k == 0), stop=(k == 5),
                            )
                        for h in range(half * 6, half * 6 + 6):
                            nc.vector.tensor_scalar(
                                out=vaug[kc][:tsz, h * 128:h * 128 + 64],
                                in0=ps[:tsz, h * 64 - half * 384:(h + 1) * 64 - half * 384],
                                scalar1=maskcol_t[:tsz, kc:kc + 1], scalar2=None,
                                op0=mybir.AluOpType.mult,
                            )
                    # mask replicated into cols [h*128+64 : h*128+128] for all heads
                    msl = maskcol_t[:tsz, kc:kc + 1]
                    mask_rep = bass.AP(
                        tensor=msl.tensor, offset=msl.offset,
                        ap=[list(msl.ap[0]), [0, H], [0, 64]],
                    )
                    dst = vaug[kc][:tsz, :].rearrange("p (h c) -> p h c", h=H)
                    nc.vector.tensor_copy(dst[:, :, 64:128], mask_rep)


                # q,k: feature-major qkvT[m*128:(m+1)*128, tokens]. Depth-2
                # software pipeline: clsfix/rope of tile i-2 is emitted during
                # tile i's qkv matmuls, so the PE queue never blocks on the
                # ACT copies of the tile it just produced.
                def emit_qkv(m):
                    t = qT[m] if m < 6 else kT6[m - 6]
                    for (qoff, qsz) in ((0, 512), (512, 512), (1024, 1)):
                        ps = qk_ps.tile([128, 512], dt.float32, name=f"qkps{m}_{qoff}", tag="qkps")
                        for k in range(6):
                            nc.tensor.matmul(
                                out=ps[:, :qsz],
                                lhsT=wq_t[k][:, m * 128:(m + 1) * 128],
                                rhs=znT[k][:, qoff:qoff + qsz],
                                start=(k == 0), stop=(k == 5),
                            )
                        nc.scalar.activation(
                            out=t[:, qoff:qoff + qsz], in_=ps[:, :qsz],
                            func=AF.Identity, bias=cvec_t[:, m:m + 1], scale=1.0,
                        )

                def emit_fixrope(m):
                    t = qT[m] if m < 6 else kT6[m - 6]
                    # CLS column: reference dots the *unpermuted* CLS q/k
                    # against roped (deinterleaved) features; undo the host
                    # deinterleave for col 0 via a PE permutation matmul
                    cps = qk_ps.tile([128, 1], dt.float32, name=f"clsps{m}", tag="clsps", bufs=2)
                    nc.tensor.matmul(out=cps, lhsT=clsP_t, rhs=t[:, 0:1])
                    # RoPE (cols 1:N): new = t*C4 + swap(t)*S4sign
                    sw = rope_p.tile([128, NP], dt.bfloat16, tag="sw")
                    nc.gpsimd.dma_start(out=sw[0:32, :], in_=t[32:64, 1:N])
                    nc.gpsimd.dma_start(out=sw[32:64, :], in_=t[0:32, 1:N])
                    nc.gpsimd.dma_start(out=sw[64:96, :], in_=t[96:128, 1:N])
                    nc.gpsimd.dma_start(out=sw[96:128, :], in_=t[64:96, 1:N])
                    ra = rope_p.tile([128, NP], dt.bfloat16, tag="ra")
                    nc.vector.tensor_mul(ra, t[:, 1:N], c4_t)
                    rb = rope_p.tile([128, NP], dt.bfloat16, tag="rb")
                    nc.vector.tensor_mul(rb, sw, s4_t)
                    if m < 6:
                        nc.vector.tensor_add(t[:, 1:N], ra, rb)
                        nc.scalar.copy(out=t[:, 0:1], in_=cps)
                    else:
                        # write roped k + fixed CLS directly into the
                        # zero-padded kTz tiles (no scatter DMAs)
                        h0 = 2 * (m - 6)
                        nc.vector.tensor_add(
                            kTz[h0][0:64, 1:N], ra[0:64, :], rb[0:64, :]
                        )
                        nc.vector.tensor_add(
                            kTz[h0 + 1][64:128, 1:N], ra[64:128, :], rb[64:128, :]
                        )
                        nc.scalar.copy(out=kTz[h0][0:64, 0:1], in_=cps[0:64, :])
                        nc.scalar.copy(
                            out=kTz[h0 + 1][64:128, 0:1], in_=cps[64:128, :]
                        )

                morder = (6, 0, 7, 1, 8, 2, 9, 3, 10, 4, 11, 5)
                for i, m in enumerate(morder):
                    emit_qkv(m)
                    if i >= 2:
                        emit_fixrope(morder[i - 2])
                emit_fixrope(morder[-2])
                emit_fixrope(morder[-1])
            znT_cm.__exit__(None, None, None)

            # ---- stage E/F/G: per-head attention ----
            # alibiT_d holds host-precomputed exp(alibi)^T, so
            # e = exp(scores) * expA — ACT reads PSUM scores directly and the
            # multiply runs in DVE 2x bf16 / GpSimd.
            with tc.tile_pool(name="alibi", bufs=6) as al_p, \
                 tc.tile_pool(name="et", bufs=30) as et_p, \
                 tc.tile_pool(name="esc", bufs=3) as esc_p, \
                 tc.tile_pool(name="spsum", bufs=3, space="PSUM") as s_ps, \
                 tc.tile_pool(name="avpsum", bufs=2, space="PSUM") as av_ps, \
                 tc.tile_pool(name="nrm", bufs=4) as nrm_p:
                # head-0 alibi prefetched before any compute so DMA streams
                # during LN/qkv
                al_pre = {}
                for kc in range(KC):
                    al = al_p.tile([128, NQ], dt.bfloat16, name=f"al0_{kc}", tag="al")
                    nc.gpsimd.dma_start(
                        out=al, in_=alibiT_d[0, kc * 128:(kc + 1) * 128, :]
                    )
                    al_pre[kc] = al

                def emit_av_mm(h, et_tiles, qi):
                    # AV matmuls (denominator = rows 64:128, replicated)
                    qoff, qsz = QCS[qi]
                    ps = av_ps.tile([128, 512], dt.float32, name=f"avps{h}_{qoff}", tag="avps")
                    for kc in range(KC):
                        nc.tensor.matmul(
                            out=ps,
                            lhsT=vaug[kc][:, h * 128:(h + 1) * 128],
                            rhs=et_tiles[kc][:, qoff:qoff + qsz],
                            start=(kc == 0), stop=(kc == KC - 1),
                        )
                    return ps

                def emit_norm(h, qi, ps, after=None):
                    # deferred so the reciprocals sit *behind* the eT mults in
                    # the DVE queue and never delay the AV-gating tiles
                    ot = outT[h // 2]
                    hh = (h % 2) * 64
                    qoff, qsz = QCS[qi]
                    rc = nrm_p.tile([64, 512], dt.float32, tag="rc")
                    ri = nc.vector.reciprocal(out=rc, in_=ps[64:128, :])
                    if after is not None:
                        add_dep_helper(ri.ins, after.ins, sync=False,
                                       reason="recip behind next head's eT mults")
                    nc.vector.tensor_mul(
                        ot[hh:hh + 64, qoff:qoff + qsz], ps[0:64, :], rc
                    )

                last_mult = [None]

                def emit_scores_range(h, kcs, et_tiles):
                    qt = qT[h // 2]
                    kt = kTz[h]
                    for kc in kcs:
                        if h == 0:
                            al = al_pre[kc]
                        else:
                            al = al_p.tile([128, NQ], dt.bfloat16, name=f"al{h}_{kc}", tag="al")
                            nc.sync.dma_start(
                                out=al, in_=alibiT_d[h, kc * 128:(kc + 1) * 128, :]
                            )
                        et = et_p.tile([128, NQ], dt.bfloat16, name=f"et{h}_{kc}", tag="et")
                        et_tiles.append(et)
                        ps = s_ps.tile([128, NQ], dt.float32, name=f"sps{h}_{kc}", tag="sps")
                        for (qoff, qsz) in QCS:
                            nc.tensor.matmul(
                                out=ps[:, qoff:qoff + qsz],
                                lhsT=kt[:, kc * 128:(kc + 1) * 128],
                                rhs=qt[:, qoff:qoff + qsz],
                            )
                        sc = esc_p.tile([128, NQ], dt.bfloat16, name=f"sc{h}_{kc}", tag="sc")
                        if kc == KC - 1:
                            # pad chunk: only key row 0 is live (vaug pad rows
                            # are zero, so eT rows 1:128 are never read)
                            if h < 3:
                                nc.gpsimd.memset(et, 0.0)
                            nc.scalar.activation(
                                out=sc[0:1, :], in_=ps[0:1, :], func=AF.Exp
                            )
                            nc.vector.tensor_mul(et[0:1, :], sc[0:1, :], al[0:1, :])
                        else:
                            nc.scalar.activation(out=sc, in_=ps, func=AF.Exp)
                            # early kc tiles can take the slow Pool engine; the
                            # last-produced ones gate the AV start, keep on DVE
                            eng = nc.gpsimd if kc in (0, 2) else nc.vector
                            last_mult[0] = eng.tensor_mul(et, sc, al)

                # depth-2 pipeline: AV(h-2) runs against eT tiles whose
                # exps finished a full head ago, so the AV stop never waits
                pipe = []
                pss = {}
                for h in range(H):
                    ets = []
                    old = pipe[0] if len(pipe) == 2 else None
                    emit_scores_range(h, range(0, 3), ets)
                    if old is not None:
                        pss[0] = emit_av_mm(old[0], old[1], 0)
                    emit_scores_range(h, range(3, 6), ets)
                    if old is not None:
                        pss[1] = emit_av_mm(old[0], old[1], 1)
                    emit_scores_range(h, range(6, KC), ets)
                    if old is not None:
                        emit_norm(old[0], 0, pss[0], after=last_mult[0])
                        emit_norm(old[0], 1, pss[1], after=last_mult[0])
                        pipe.pop(0)
                    pipe.append((h, ets))
                for (h, ets) in pipe:
                    emit_norm(h, 0, emit_av_mm(h, ets, 0))
                    emit_norm(h, 1, emit_av_mm(h, ets, 1))

            # ---- host-path dumps (no dependents; gpsimd DMA queue so the
            # alibi stream on the sync queue is never blocked) ----
            for m in range(6):
                nc.gpsimd.dma_start(
                    out=qdump_d[m * 128:(m + 1) * 128, :], in_=qT[m][:, NQ:N]
                )
            for h in range(H):
                hh = (h % 2) * 64
                nc.gpsimd.dma_start(
                    out=kdump_d[h * 64:(h + 1) * 64, :],
                    in_=kTz[h][hh:hh + 64, 0:N],
                )
            for kc in range(KC):
                src = vaug[kc].rearrange("p (h c) -> p h c", h=H)
                nc.gpsimd.dma_start(
                    out=vdump_d[kc * 128:(kc + 1) * 128, :].rearrange(
                        "p (h c) -> p h c", h=H),
                    in_=src[:, :, 0:64],
                )

            # ---- stage H: out projection (tokens 0:1024) ----
            with tc.tile_pool(name="opsum", bufs=4, space="PSUM") as o_ps, \
                 tc.tile_pool(name="osb", bufs=2) as osb_p:
                for (toff, tsz) in OCS:
                    ob = osb_p.tile([128, D], dt.float32)
                    for nn2 in range(2):
                        ps = o_ps.tile([128, 384], dt.float32)
                        for k in range(6):
                            nc.tensor.matmul(
                                out=ps,
                                lhsT=outT[k][:, toff:toff + tsz],
                                rhs=wout_t[k][:, nn2 * 384:(nn2 + 1) * 384],
                                start=(k == 0), stop=(k == 5),
                            )
                        nc.scalar.copy(
                            out=ob[:, nn2 * 384:(nn2 + 1) * 384], in_=ps
                        )
                    nc.sync.dma_start(out=out_d[toff:toff + tsz, :], in_=ob)

    _split_oversized_waits(nc)
    return nc


def _split_oversized_waits(nc):
    """Walrus rejects >1 sync wait per instruction; hoist extras onto NoOps."""
    import bass_rust
    for f in nc.m.functions:
        for bb in f.blocks:
            il = bb.instructions
            i = 0
            while i < len(il):
                inst = il[i]
                si = inst.sync_info
                if si is not None and si.on_wait and len(si.on_wait) > 1:
                    waits = list(si.on_wait)
                    inst.sync_info = bass_rust.SyncInfo(
                        on_wait=[waits[-1]], on_update=list(si.on_update)
                    )
                    pos = i
                    for j, w in enumerate(waits[:-1]):
                        n = bass_rust.InstNoOp(name=f"{inst.name}-wsplit{j}")
                        n.engine = inst.engine
                        n.sync_info = bass_rust.SyncInfo(on_wait=[w], on_update=[])
                        il.insert(pos, n)
                        pos += 1
                        i += 1
                i += 1


def _host_prep(x, alibi_bias, coords, mask, gamma, beta, W_qkv, W_out):
    """Build per-core input maps (host-side weight prep + sharding)."""
    x = np.asarray(x, np.float32)
    alibi = np.asarray(alibi_bias, np.float32)[0]          # [H, N, N]
    coords = np.asarray(coords, np.float32)
    mask = np.asarray(mask).astype(np.float32)             # [B, N]
    gamma = np.asarray(gamma, np.float32)
    beta = np.asarray(beta, np.float32)
    W_qkv = np.asarray(W_qkv, np.float32)
    W_out = np.asarray(W_out, np.float32)

    # deinterleave rope pairs in q,k head blocks; fold scale into q; gamma into W
    perm = np.arange(3 * D)
    de = np.concatenate([np.arange(0, DH, 2), np.arange(1, DH, 2)])
    for h in range(H):
        perm[h * DH:(h + 1) * DH] = h * DH + de
        perm[D + h * DH:D + (h + 1) * DH] = D + h * DH + de
    Wp = W_qkv[:, perm].copy()
    Wp[:, :D] *= SCALE
    cvec = (beta @ Wp).astype(np.float32).reshape(3 * D, 1)
    Wp = (gamma[:, None] * Wp).astype(BF16)
    Wo = W_out.astype(BF16)

    # exp(alibi): transpose to [H, key, query 0:1024], pad keys to NK with 0
    # (e = exp(scores)*expA, so pad keys contribute exactly 0)
    alibiT = np.zeros((H, NK, NQ), dtype=BF16)
    alibiT[:, :N, :] = np.exp(alibi[:, :NQ, :]).transpose(0, 2, 1)

    # CLS un-deinterleave permutation as a PE matmul weight:
    # out[hh+2r] = in[hh+r], out[hh+2r+1] = in[hh+32+r]; lhsT[k,m] = P[m,k]
    clsP = np.zeros((128, 128), dtype=BF16)
    for hh in (0, 64):
        for r in range(32):
            clsP[hh + r, hh + 2 * r] = 1
            clsP[hh + 32 + r, hh + 2 * r + 1] = 1

    # rope tables per batch: stacked [cos;cos;cos;cos], [-sin;sin;-sin;sin]
    inv_freq = 1.0 / (ROPE_BASE ** (np.arange(HALF, dtype=np.float32) / HALF))
    in_maps = []
    for b in range(B):
        xy = coords[b, :, 0] + coords[b, :, 1]             # [NP]
        fr = inv_freq[:, None] * xy[None, :]               # [HALF, NP]
        c, s = np.cos(fr), np.sin(fr)
        c4 = np.tile(c, (4, 1)).astype(BF16)               # [128, NP]
        s4 = np.concatenate([-s, s, -s, s], 0).astype(BF16)
        maskcol = np.zeros((NK, 1), np.float32)
        maskcol[:N, 0] = mask[b]
        in_maps.append({
            "x": x[b],
            "clsP": clsP,
            "alibiT": alibiT,
            "wqkv": Wp,
            "wout": Wo,
            "cvec": cvec,
            "maskcol": maskcol,
            "c4": c4,
            "s4": s4,
        })
    return in_maps


def _host_row1024(res, b, alibi, mask, W_out):
    """Finish query token 1024 on host from device dumps (fp32)."""
    r = res.results[b]
    q = r["qdump"][:, 0].astype(np.float32)                # [768] roped q_1024
    K = r["kdump"].astype(np.float32)                      # [768, 1025]
    Vp = r["vdump"][:N, :].astype(np.float32)              # [1025, 768] masked v
    out = np.empty(D, np.float32)
    acc = np.zeros(D, np.float32)
    for h in range(H):
        qh = q[h * DH:(h + 1) * DH]
        Kh = K[h * DH:(h + 1) * DH, :]                     # [64, 1025]
        s = qh @ Kh + alibi[h, NQ, :]                      # [1025]
        e = np.exp(s) * mask                               # masked exp weights
        den = e.sum()
        num = e @ Vp[:, h * DH:(h + 1) * DH]               # [64]
        acc[h * DH:(h + 1) * DH] = num / den
    return acc @ W_out


def kernel(x, alibi_bias, coords, mask, gamma, beta, W_qkv, W_out):
    global LAST_RESULTS
    from concourse.bass_utils import run_bass_kernel_spmd

    if "nc" not in _CACHE:
        _CACHE["nc"] = _build_program()
    nc = _CACHE["nc"]

    in_maps = _host_prep(x, alibi_bias, coords, mask, gamma, beta, W_qkv, W_out)
    res = run_bass_kernel_spmd(nc, in_maps, list(range(B)))
    LAST_RESULTS = res

    alibi = np.asarray(alibi_bias, np.float32)[0]
    maskf = np.asarray(mask).astype(np.float32)
    Wo = np.asarray(W_out, np.float32)
    out = np.empty((B, N, D), dtype=np.float32)
    for b in range(B):
        out[b, :NQ] = res.results[b]["out"]
        out[b, NQ] = _host_row1024(res, b, alibi, maskf[b], Wo)
    return out
